# revision 1
# baseline (speedup 1.0000x reference)
"""Trainium2 Bass kernel for nn_NeuralTrustNetwork (gnn_message_passing).

out[e] = lrelu(lrelu(c) @ W_mlp + b_mlp) @ wL + bL
         + (x[src]*x[dst]) @ w1 + b1 + (w[src]*w[dst]) @ w2 + b2
  with c = (s1+s2)[src] + (p1+p2)[dst]

Strategy (edge-parallel across 8 NeuronCores, per the sharding hint):
- Host: build combined fp16 node tables SRC=[s1|s2|x|w], DST=[p1|p2|x|w]
  (512B rows), split into lo/hi halves (dma_gather indices are int16),
  bucket each core's edges by (src-half, dst-half), pad buckets to
  2048-edge batches.
- Device per 2048-edge batch: 4x dma_gather (1024 rows each — SWDGE
  descriptor-ring limit), DVE adds for c, PE pair-transposes + blockdiag
  W_mlp matmul for the MLP, fused LeakyReLU+bias on ACT, dot-product
  heads on DVE, everything accumulated into a [16,128] PSUM tile that
  stores contiguously.
"""

from contextlib import ExitStack

import numpy as np

import concourse.bacc as bacc
import concourse.bass as bass
import concourse.mybir as mybir
import concourse.tile as tile
from concourse.masks import make_identity

FP16 = mybir.dt.float16
F32 = mybir.dt.float32
I16 = mybir.dt.int16

NCORES = 8
B = 2048          # edges per compute batch
BG = 1024         # edges per dma_gather (descriptor ring limit)
J = B // 128      # 16 slots
D = 64

_prog_cache = {}

# gather position i -> DRAM/out position q: q = (i%128)*16 + 8*(i//1024) + (i//128)%8
_I = np.arange(B)
_QPERM = (_I % 128) * (B // 128) + (B // 1024) * 4 * (_I // 1024) + (_I // 128) % 8
# inverse: gather list position i must hold stream edge (base + _QPERM[i])


def _wrap_idx_chunk(idx):
    """[1024] int -> [128, 64] int16 (wrap 16 partitions, replicate 8x)."""
    w = idx.reshape(-1, 16).T.astype(np.int16)  # [16, 64]
    return np.tile(w, (8, 1))


def _build_program(nb, nhalf, reps=1, variant='full'):
    """Build + compile the SPMD program for per-bucket batch counts nb[0..3]."""
    totb = sum(nb)
    nc = bacc.Bacc(
        "TRN2",
        target_bir_lowering=False,
        debug=False,
        enable_asserts=False,
        num_swdge_queues=4,
    )
    src_lo = nc.dram_tensor("src_lo", [nhalf, 256], FP16, kind="ExternalInput").ap()
    src_hi = nc.dram_tensor("src_hi", [nhalf, 256], FP16, kind="ExternalInput").ap()
    dst_lo = nc.dram_tensor("dst_lo", [nhalf, 256], FP16, kind="ExternalInput").ap()
    dst_hi = nc.dram_tensor("dst_hi", [nhalf, 256], FP16, kind="ExternalInput").ap()
    idx_s = nc.dram_tensor("idx_s", [totb * 128, 128], I16, kind="ExternalInput").ap()
    idx_d = nc.dram_tensor("idx_d", [totb * 128, 128], I16, kind="ExternalInput").ap()
    wbd_d = nc.dram_tensor("wbd", [128, 128], FP16, kind="ExternalInput").ap()
    wlp_d = nc.dram_tensor("wlp", [128, 2], FP16, kind="ExternalInput").ap()
    bm2_d = nc.dram_tensor("bm2", [128, 1], F32, kind="ExternalInput").ap()
    w12_d = nc.dram_tensor("w12", [1, 2048], FP16, kind="ExternalInput").ap()
    b3_d = nc.dram_tensor("b3", [1, 3], F32, kind="ExternalInput").ap()
    out_d = nc.dram_tensor("out", [totb * 128, 16], F32, kind="ExternalOutput").ap()

    s_tabs = [src_lo, src_lo, src_hi, src_hi]
    d_tabs = [dst_lo, dst_hi, dst_lo, dst_hi]

    with tile.TileContext(nc) as tc, ExitStack() as ctx:
        const = ctx.enter_context(tc.tile_pool(name="const", bufs=1))
        ident = const.tile([128, 128], FP16)
        make_identity(nc, ident[:])
        wbd_t = const.tile([128, 128], FP16)
        nc.sync.dma_start(wbd_t[:], wbd_d[:])
        wlp_t = const.tile([128, 2], FP16)
        nc.sync.dma_start(wlp_t[:], wlp_d[:])
        bm2_t = const.tile([128, 1], F32)
        nc.sync.dma_start(bm2_t[:], bm2_d[:])
        w12r_t = const.tile([1, 2048], FP16)
        nc.sync.dma_start(w12r_t[:], w12_d[:])
        b3_t = const.tile([1, 3], F32)
        nc.sync.dma_start(b3_t[:], b3_d[:])
        ones1 = const.tile([1, 128], FP16)
        nc.vector.memset(ones1[:], 1.0)
        ones128f = const.tile([1, 128], F32)
        nc.vector.memset(ones128f[:], 1.0)

        w12big = const.tile([128, 2048], FP16)
        k128 = const.tile([128, 1], F32)
        with tc.tile_pool(name="setup_ps", bufs=1, space="PSUM") as sps:
            for q in range(4):
                pw = sps.tile([128, 512], F32)
                nc.tensor.matmul(
                    pw[:], lhsT=ones1[:], rhs=w12r_t[:, q * 512:(q + 1) * 512],
                    start=True, stop=True,
                )
                nc.scalar.copy(w12big[:, q * 512:(q + 1) * 512], pw[:])
            pk = sps.tile([128, 3], F32)
            nc.tensor.matmul(pk[:], lhsT=ones128f[:], rhs=b3_t[:], start=True, stop=True)
            nc.vector.tensor_reduce(k128[:], pk[:], axis=mybir.AxisListType.X,
                                    op=mybir.AluOpType.add)

        idxp = ctx.enter_context(tc.tile_pool(name="idx", bufs=3))
        gp = ctx.enter_context(tc.tile_pool(name="gath", bufs=3))
        cp = ctx.enter_context(tc.tile_pool(name="csum", bufs=2))
        atp = ctx.enter_context(tc.tile_pool(name="at", bufs=4))
        l2p = ctx.enter_context(tc.tile_pool(name="l2", bufs=4))
        mp = ctx.enter_context(tc.tile_pool(name="m2", bufs=2))
        outp = ctx.enter_context(tc.tile_pool(name="outs", bufs=3))
        ps_c = ctx.enter_context(tc.tile_pool(name="ps_c", bufs=2, space="PSUM"))
        ps_h = ctx.enter_context(tc.tile_pool(name="ps_h", bufs=2, space="PSUM"))
        ps_o = ctx.enter_context(tc.tile_pool(name="ps_o", bufs=2, space="PSUM"))

        for rep in range(reps):
          t = 0
          for k in range(4):
            s_tab, d_tab = s_tabs[k], d_tabs[k]
            for _ in range(nb[k]):
                si = idxp.tile([128, 128], I16, tag="si")
                nc.sync.dma_start(si[:], idx_s[t * 128:(t + 1) * 128, :])
                di = idxp.tile([128, 128], I16, tag="di")
                nc.sync.dma_start(di[:], idx_d[t * 128:(t + 1) * 128, :])

                S = gp.tile([128, J, 256], FP16, tag="S")
                Dt = gp.tile([128, J, 256], FP16, tag="D")
                for h in range(2 if variant != 'compute' else 0):
                    nc.gpsimd.dma_gather(
                        out_ap=S[:, h * 8:(h + 1) * 8, :], in_ap=s_tab[:],
                        idxs_ap=si[:, h * 64:(h + 1) * 64],
                        num_idxs=BG, num_idxs_reg=BG, elem_size=256,
                        queue_num=2 * h,
                    )
                    nc.gpsimd.dma_gather(
                        out_ap=Dt[:, h * 8:(h + 1) * 8, :], in_ap=d_tab[:],
                        idxs_ap=di[:, h * 64:(h + 1) * 64],
                        num_idxs=BG, num_idxs_reg=BG, elem_size=256,
                        queue_num=2 * h + 1,
                    )

                if variant == 'gather':
                    t += 1
                    continue
                # c = (s1+s2)[src] + (p1+p2)[dst]   [128, J, 64] fp16
                t1 = cp.tile([128, J, D], FP16, tag="t1")
                nc.vector.tensor_tensor(t1[:], S[:, :, 0:64], S[:, :, 64:128],
                                        op=mybir.AluOpType.add)
                t2 = cp.tile([128, J, D], FP16, tag="t2")
                nc.vector.tensor_tensor(t2[:], Dt[:, :, 0:64], Dt[:, :, 64:128],
                                        op=mybir.AluOpType.add)
                c = cp.tile([128, J, D], FP16, tag="c")
                nc.vector.tensor_tensor(c[:], t1[:], t2[:], op=mybir.AluOpType.add)

                # dot-product heads: m2 = sum_d x_s*x_d*w1 + w_s*w_d*w2  [128, J]
                tmp = mp.tile([128, J, 128], FP16, tag="tmp")
                nc.vector.tensor_tensor(tmp[:], S[:, :, 128:256],
                                        w12big[:].rearrange("p (j e) -> p j e", j=J),
                                        op=mybir.AluOpType.mult)
                tmp2 = mp.tile([128, J, 128], FP16, tag="tmp2")
                nc.vector.tensor_tensor(tmp2[:], tmp[:], Dt[:, :, 128:256],
                                        op=mybir.AluOpType.mult)
                m2 = mp.tile([128, J], F32, tag="m2")
                nc.vector.tensor_reduce(m2[:], tmp2[:], axis=mybir.AxisListType.X,
                                        op=mybir.AluOpType.add)
                # MLP path per slot-pair; e1 accumulates edge-major [128, J]
                e1 = ps_o.tile([128, J], F32)
                for u in range(8):
                    pc = ps_c.tile([128, 128], FP16, tag="pc")
                    nc.tensor.matmul(pc[:], lhsT=c[:, 2 * u:2 * u + 2, :],
                                     rhs=ident[:], is_transpose=True,
                                     start=True, stop=True)
                    at = atp.tile([128, 128], FP16, tag="at")
                    nc.scalar.activation(at[:], pc[:],
                                         mybir.ActivationFunctionType.Lrelu,
                                         alpha=0.01)
                    ph = ps_h.tile([128, 128], F32, tag="ph")
                    nc.tensor.matmul(ph[:], lhsT=wbd_t[:], rhs=at[:],
                                     start=True, stop=True)
                    l2 = l2p.tile([128, 128], FP16, tag="l2")
                    nc.scalar.activation(l2[:], ph[:],
                                         mybir.ActivationFunctionType.Lrelu,
                                         bias=bm2_t[:, 0:1], alpha=0.01)
                    nc.tensor.matmul(e1[:, 2 * u:2 * u + 2], lhsT=l2[:],
                                     rhs=wlp_t[:], start=True, stop=True)

                ot = outp.tile([128, J], F32)
                nc.vector.tensor_tensor(ot[:], e1[:], m2[:], op=mybir.AluOpType.add)
                nc.vector.tensor_scalar_add(ot[:], ot[:], k128[:, 0:1])
                nc.sync.dma_start(out_d[t * 128:(t + 1) * 128, :], ot[:])
                t += 1

    nc.compile()
    return nc


def _prep(inputs):
    src = np.asarray(inputs["src"]).astype(np.int64).ravel()
    dst = np.asarray(inputs["dst"]).astype(np.int64).ravel()
    s1 = np.asarray(inputs["s1"], np.float32)
    s2 = np.asarray(inputs["s2"], np.float32)
    p1 = np.asarray(inputs["p1"], np.float32)
    p2 = np.asarray(inputs["p2"], np.float32)
    x = np.asarray(inputs["x"], np.float32)
    w = np.asarray(inputs["w"], np.float32)

    E = src.shape[0]
    N = s1.shape[0]
    assert E % NCORES == 0
    epc = E // NCORES
    nhalf = (N + 1) // 2

    src_tab = np.concatenate([s1, s2, x, w], axis=1).astype(np.float16)
    dst_tab = np.concatenate([p1, p2, x, w], axis=1).astype(np.float16)
    if N < 2 * nhalf:
        padrow = np.zeros((2 * nhalf - N, 256), np.float16)
        src_tab = np.vstack([src_tab, padrow])
        dst_tab = np.vstack([dst_tab, padrow])

    # bucket per core
    per_core = []
    counts = np.zeros((NCORES, 4), np.int64)
    for c in range(NCORES):
        s = src[c * epc:(c + 1) * epc]
        d = dst[c * epc:(c + 1) * epc]
        b = (s >= nhalf) * 2 + (d >= nhalf)
        ords = [np.flatnonzero(b == k) for k in range(4)]
        counts[c] = [len(o) for o in ords]
        per_core.append((s, d, ords))

    nb = [int(-(-counts[:, k].max() // B)) for k in range(4)]
    totb = sum(nb)

    idx_s_all = np.zeros((NCORES, totb * 128, 128), np.int16)
    idx_d_all = np.zeros((NCORES, totb * 128, 128), np.int16)
    order_all = np.full((NCORES, totb * B), -1, np.int64)

    for c in range(NCORES):
        s, d, ords = per_core[c]
        t = 0
        pos = 0
        for k in range(4):
            ids = ords[k]
            cap = nb[k] * B
            se = np.zeros(cap, np.int64)
            de = np.zeros(cap, np.int64)
            se[:len(ids)] = s[ids] - (nhalf if k >= 2 else 0)
            de[:len(ids)] = d[ids] - (nhalf if k % 2 == 1 else 0)
            order_all[c, pos:pos + len(ids)] = ids
            pos += cap
            for bi in range(nb[k]):
                blk_s = np.empty((128, 128), np.int16)
                blk_d = np.empty((128, 128), np.int16)
                seg_s = se[bi * B + _QPERM]
                seg_d = de[bi * B + _QPERM]
                for h in range(2):
                    sl = slice(h * BG, (h + 1) * BG)
                    blk_s[:, h * 64:(h + 1) * 64] = _wrap_idx_chunk(seg_s[sl])
                    blk_d[:, h * 64:(h + 1) * 64] = _wrap_idx_chunk(seg_d[sl])
                idx_s_all[c, t * 128:(t + 1) * 128] = blk_s
                idx_d_all[c, t * 128:(t + 1) * 128] = blk_d
                t += 1

    # weights
    W_mlp = np.asarray(inputs["W_mlp"], np.float32)
    b_mlp = np.asarray(inputs["b_mlp"], np.float32).ravel()
    wL = np.asarray(inputs["wL"], np.float32).ravel()
    w1 = np.asarray(inputs["w1"], np.float32).ravel()
    w2 = np.asarray(inputs["w2"], np.float32).ravel()
    bL = float(np.asarray(inputs["bL"]).ravel()[0])
    b1 = float(np.asarray(inputs["b1"]).ravel()[0])
    b2 = float(np.asarray(inputs["b2"]).ravel()[0])

    wbd = np.zeros((128, 128), np.float16)
    wbd[:64, :64] = W_mlp.astype(np.float16)
    wbd[64:, 64:] = W_mlp.astype(np.float16)
    wlp = np.zeros((128, 2), np.float16)
    wlp[:64, 0] = wL.astype(np.float16)
    wlp[64:, 1] = wL.astype(np.float16)
    bm2 = np.concatenate([b_mlp, b_mlp]).astype(np.float32).reshape(128, 1)
    w12 = np.tile(np.concatenate([w1, w2]).astype(np.float16), J).reshape(1, 2048)
    b3 = np.array([[bL, b1, b2]], np.float32)

    weights = dict(wbd=wbd, wlp=wlp, bm2=bm2, w12=w12, b3=b3)
    tabs = dict(
        src_lo=np.ascontiguousarray(src_tab[:nhalf]),
        src_hi=np.ascontiguousarray(src_tab[nhalf:]),
        dst_lo=np.ascontiguousarray(dst_tab[:nhalf]),
        dst_hi=np.ascontiguousarray(dst_tab[nhalf:]),
    )
    return (tuple(nb), nhalf, epc, E, tabs, weights,
            idx_s_all, idx_d_all, order_all)


def run(inputs, **spmd_kwargs):
    """Returns (output [E,1] float32, BassKernelResults)."""
    from concourse.bass_utils import run_bass_kernel_spmd

    (nb, nhalf, epc, E, tabs, weights,
     idx_s_all, idx_d_all, order_all) = _prep(inputs)

    key = (nb, nhalf)
    if key not in _prog_cache:
        _prog_cache[key] = _build_program(list(nb), nhalf)
    nc = _prog_cache[key]

    in_maps = []
    for c in range(NCORES):
        m = dict(tabs)
        m.update(weights)
        m["idx_s"] = idx_s_all[c]
        m["idx_d"] = idx_d_all[c]
        in_maps.append(m)

    res = run_bass_kernel_spmd(nc, in_maps, list(range(NCORES)), **spmd_kwargs)

    out = np.empty((E, 1), np.float32)
    for c in range(NCORES):
        oc = np.asarray(res.results[c]["out"], np.float32).reshape(-1)
        order = order_all[c]
        valid = order >= 0
        out[c * epc + order[valid], 0] = oc[valid]
    return out, res


def kernel(**inputs) -> np.ndarray:
    out, _ = run(inputs)
    return out



# revision 2
# speedup vs baseline: 795.3209x; 795.3209x over previous
"""Trainium2 Bass kernel v2 for nn_NeuralTrustNetwork (gnn_message_passing).

out[e] = lrelu(lrelu(c) @ W_mlp + b_mlp) @ wL + bL
         + (x[src]*x[dst]) @ w1 + b1 + (w[src]*w[dst]) @ w2 + b2
  with c = (s1+s2)[src] + (p1+p2)[dst]

v2 strategy (vs v1 which gathered BOTH endpoints per edge):
- Host sorts edges by src and shards contiguous ranges across the 8 cores,
  then splits each core's edges by dst half (int16 gather indices) keeping
  src order, and cuts batches of <=1024 edges spanning <=128 consecutive
  src ids.
- src rows are NOT gathered per edge. The host ships a dense 128-row
  window per batch (win_tab); the device expands per-edge src rows with
  one-hot matmuls on the (otherwise idle) PE, accumulating the dst-row add
  for the MLP input directly in PSUM. Head weights w1/w2 are folded into
  the src window table; s1+s2 / p1+p2 are pre-summed on host.
- dst rows (512B) are gathered per edge via SWDGE on all 4 queues.
  This halves the random-gather HBM traffic, which is the bottleneck.
"""

from contextlib import ExitStack

import numpy as np

import concourse.bacc as bacc
import concourse.bass as bass
import concourse.mybir as mybir
import concourse.tile as tile
from concourse.masks import make_identity

FP16 = mybir.dt.float16
F32 = mybir.dt.float32
I16 = mybir.dt.int16

NCORES = 8
B = 1024            # edges per batch
J = B // 128        # 8 slots
W = 128             # src window size per batch
D = 64
N_USERS = 50000
NHALF = 25000
PADVAL = 200.0      # winrel pad: never matches iota 0..127

_prog_cache = {}


def _wrap_idx(idx):
    """[1024] int -> [128, 64] int16 (wrap 16 partitions, replicate 8x)."""
    w = np.asarray(idx).reshape(-1, 16).T.astype(np.int16)
    return np.tile(w, (8, 1))


def _build_program(nb_lo, nb_hi, reps=1, variant='full', bcast='pe',
                   gbufs=8, nchunk=4, skip=()):
    nbt = nb_lo + nb_hi
    assert nbt <= 128
    nc = bacc.Bacc(
        "TRN2",
        target_bir_lowering=False,
        debug=False,
        enable_asserts=False,
        num_swdge_queues=4,
    )
    # transposed/packed layouts, preloaded to SBUF at setup:
    # win_tab_t[p, b*192:(b+1)*192] = src window row p of batch b
    # winrel_t[b, :]  = per-edge window-relative src ids of batch b
    # dstidx_t[p, b*64:(b+1)*64] = wrapped gather indices of batch b
    nbt4 = -(-nbt // 4)
    win_tab = nc.dram_tensor("win_tab", [128, nbt * 192], FP16,
                             kind="ExternalInput").ap()
    winrel = nc.dram_tensor("winrel", [nbt4, 4 * B], FP16,
                            kind="ExternalInput").ap()
    dstidx = nc.dram_tensor("dstidx", [128, nbt * 64], I16,
                            kind="ExternalInput").ap()
    dst_lo = nc.dram_tensor("dst_lo", [NHALF, 256], FP16,
                            kind="ExternalInput").ap()
    dst_hi = nc.dram_tensor("dst_hi", [N_USERS - NHALF, 256], FP16,
                            kind="ExternalInput").ap()
    wbd_d = nc.dram_tensor("wbd", [128, 128], FP16, kind="ExternalInput").ap()
    wlp_d = nc.dram_tensor("wlp", [128, 2], FP16, kind="ExternalInput").ap()
    bm2_d = nc.dram_tensor("bm2", [128, 1], F32, kind="ExternalInput").ap()
    b3_d = nc.dram_tensor("b3", [1, 3], F32, kind="ExternalInput").ap()
    out_d = nc.dram_tensor("out", [nbt * 128, J], F32,
                           kind="ExternalOutput").ap()

    with tile.TileContext(nc) as tc, ExitStack() as ctx:
        const = ctx.enter_context(tc.tile_pool(name="const", bufs=1))
        ident = const.tile([128, 128], FP16)
        make_identity(nc, ident[:])
        wbd_t = const.tile([128, 128], FP16)
        nc.sync.dma_start(wbd_t[:], wbd_d[:])
        wlp_t = const.tile([128, 2], FP16)
        nc.sync.dma_start(wlp_t[:], wlp_d[:])
        bm2_t = const.tile([128, 1], F32)
        nc.sync.dma_start(bm2_t[:], bm2_d[:])
        b3_t = const.tile([1, 3], F32)
        nc.sync.dma_start(b3_t[:], b3_d[:])
        ones128f = const.tile([1, 128], F32)
        nc.vector.memset(ones128f[:], 1.0)
        ones1h = const.tile([1, 128], FP16)
        nc.vector.memset(ones1h[:], 1.0)

        iota16 = const.tile([128, 1], I16)
        nc.gpsimd.iota(iota16[:], pattern=[[1, 1]], base=0,
                       channel_multiplier=1)
        iota_h = const.tile([128, 1], FP16)
        nc.scalar.copy(iota_h[:], iota16[:])
        iota_f = const.tile([128, 1], F32)
        nc.scalar.copy(iota_f[:], iota16[:])

        k128 = const.tile([128, 1], F32)
        with tc.tile_pool(name="setup_ps", bufs=1, space="PSUM") as sps:
            pk = sps.tile([128, 3], F32)
            nc.tensor.matmul(pk[:], lhsT=ones128f[:], rhs=b3_t[:],
                             start=True, stop=True)
            nc.vector.tensor_reduce(k128[:], pk[:], axis=mybir.AxisListType.X,
                                    op=mybir.AluOpType.add)

        # preload all per-batch inputs (no HWDGE traffic during gathers)
        pre = ctx.enter_context(tc.tile_pool(name="pre", bufs=1))
        cb = -(-nbt // nchunk)  # batches per chunk
        idx_ch = []
        wt_ch = []
        for ci in range(nchunk):
            lo, hi = ci * cb, min((ci + 1) * cb, nbt)
            if lo >= hi:
                idx_ch.append(None)
                wt_ch.append(None)
                continue
            it = pre.tile([128, (hi - lo) * 64], I16, tag=f"ic{ci}")
            nc.sync.dma_start(it[:], dstidx[:, lo * 64:hi * 64])
            idx_ch.append((it, lo))
            wtt = pre.tile([128, (hi - lo) * 192], FP16, tag=f"wc{ci}")
            nc.sync.dma_start(wtt[:], win_tab[:, lo * 192:hi * 192])
            wt_ch.append((wtt, lo))
        wrp = ctx.enter_context(tc.tile_pool(name="wrp", bufs=3))

        gp = ctx.enter_context(tc.tile_pool(name="gath", bufs=gbufs))
        ohp = ctx.enter_context(tc.tile_pool(name="oh", bufs=3))
        atp = ctx.enter_context(tc.tile_pool(name="at", bufs=2))
        a2p = ctx.enter_context(tc.tile_pool(name="at2", bufs=2))
        l2p = ctx.enter_context(tc.tile_pool(name="l2", bufs=2))
        prp = ctx.enter_context(tc.tile_pool(name="pr", bufs=2))
        mp = ctx.enter_context(tc.tile_pool(name="m2", bufs=2))
        outp = ctx.enter_context(tc.tile_pool(name="outs", bufs=3))
        ps_wr = ctx.enter_context(tc.tile_pool(name="ps_wr", bufs=1,
                                               space="PSUM"))
        ps_c = ctx.enter_context(tc.tile_pool(name="ps_c", bufs=1,
                                              space="PSUM"))
        ps_xw = ctx.enter_context(tc.tile_pool(name="ps_xw", bufs=1,
                                               space="PSUM"))
        ps_pc = ctx.enter_context(tc.tile_pool(name="ps_pc", bufs=1,
                                               space="PSUM"))
        ps_h = ctx.enter_context(tc.tile_pool(name="ps_h", bufs=1,
                                              space="PSUM"))
        ps_o = ctx.enter_context(tc.tile_pool(name="ps_o", bufs=1,
                                              space="PSUM"))

        def slices(b):
            ic, ilo = idx_ch[b // cb]
            wc, wlo = wt_ch[b // cb]
            return (ic[:, (b - ilo) * 64:(b - ilo + 1) * 64],
                    wc[:, (b - wlo) * 192:(b - wlo + 1) * 192])

        for rep in range(reps):
            if variant == 'gather':
                for b in range(nbt):
                    dtab = dst_lo if b < nb_lo else dst_hi
                    di, wt = slices(b)
                    Dt = gp.tile([128, J, 256], FP16, tag="D")
                    nc.gpsimd.dma_gather(
                        out_ap=Dt[:], in_ap=dtab[:], idxs_ap=di,
                        num_idxs=B, num_idxs_reg=B, elem_size=256,
                        queue_num=b % 4,
                    )
                    ot = outp.tile([128, J], F32)
                    nc.vector.tensor_reduce(
                        ot[:], Dt[:], axis=mybir.AxisListType.X,
                        op=mybir.AluOpType.add)
                    nc.sync.dma_start(out_d[b * 128:(b + 1) * 128, :], ot[:])
                continue

            # software-pipelined emission: iteration i runs
            #   early(i+1): winrel load, PE broadcast, is_equal -> oh
            #   mid(i):     expansion matmuls, at-lrelu, head products
            #   late(i-1):  transposes, W matmul, lrelu2, e1, out
            st = {}   # b -> dict of live tiles
            wr4 = None

            def early(b):
                nonlocal wr4
                if b % 4 == 0:
                    wr4 = wrp.tile([1, 4 * B], FP16, tag="wr4")
                    nc.sync.dma_start(wr4[:], winrel[b // 4:b // 4 + 1, :])
                wrb = ps_wr.tile([128, B], F32)
                for k in range(2):
                    nc.tensor.matmul(
                        wrb[:, k * 512:(k + 1) * 512], lhsT=ones1h[:],
                        rhs=wr4[:, (b % 4) * B + k * 512:
                                (b % 4) * B + (k + 1) * 512],
                        start=True, stop=True)
                oh = ohp.tile([128, B], FP16, tag="oh")
                nc.vector.tensor_scalar(
                    oh[:], wrb[:], iota_f[:, 0:1], None,
                    op0=mybir.AluOpType.is_equal)
                st[b] = {'oh': oh}

            def mid(b):
                s = st[b]
                di, wt = slices(b)
                oh = s['oh']
                Dt = s['Dt']
                psc = ps_c.tile([128, J, D], F32)
                for u in range(J):
                    nc.tensor.matmul(psc[:, u, :],
                                     lhsT=oh[:, u * 128:(u + 1) * 128],
                                     rhs=wt[:, 0:64], start=True, stop=False)
                    nc.tensor.matmul(psc[:, u, :], lhsT=ident[:],
                                     rhs=Dt[:, u, 0:64],
                                     start=False, stop=True)
                psxw = ps_xw.tile([128, J, 128], F32)
                for u in range(J):
                    nc.tensor.matmul(psxw[:, u, :],
                                     lhsT=oh[:, u * 128:(u + 1) * 128],
                                     rhs=wt[:, 64:192], start=True, stop=True)
                at = atp.tile([128, J, D], FP16, tag="at")
                nc.scalar.activation(at[:], psc[:],
                                     mybir.ActivationFunctionType.Lrelu,
                                     alpha=0.01)
                pr = prp.tile([128, J, 128], FP16, tag="pr")
                nc.vector.tensor_tensor(pr[:], psxw[:], Dt[:, :, 64:192],
                                        op=mybir.AluOpType.mult)
                m2 = mp.tile([128, J], F32, tag="m2")
                nc.vector.tensor_reduce(m2[:], pr[:],
                                        axis=mybir.AxisListType.X,
                                        op=mybir.AluOpType.add)
                s.update(at=at, m2=m2)

            ot4 = None

            def late(b):
                nonlocal ot4
                s = st.pop(b)
                at, m2 = s['at'], s['m2']
                pc = ps_pc.tile([128, J // 2, 128], FP16)
                for q in range(J // 2):
                    nc.tensor.matmul(pc[:, q, :],
                                     lhsT=at[:, 2 * q:2 * q + 2, :],
                                     rhs=ident[:], is_transpose=True,
                                     start=True, stop=True)
                at2 = a2p.tile([128, J // 2, 128], FP16, tag="at2")
                nc.scalar.copy(at2[:], pc[:])
                ph = ps_h.tile([128, J // 2, 128], F32)
                for q in range(J // 2):
                    nc.tensor.matmul(ph[:, q, :], lhsT=wbd_t[:],
                                     rhs=at2[:, q, :], start=True, stop=True)
                l2 = l2p.tile([128, J // 2, 128], FP16, tag="l2")
                nc.scalar.activation(l2[:], ph[:],
                                     mybir.ActivationFunctionType.Lrelu,
                                     bias=bm2_t[:, 0:1], alpha=0.01)
                e1 = ps_o.tile([128, J], F32)
                for q in range(J // 2):
                    nc.tensor.matmul(e1[:, 2 * q:2 * q + 2],
                                     lhsT=l2[:, q, :], rhs=wlp_t[:],
                                     start=True, stop=True)
                if b % 4 == 0:
                    ot4 = outp.tile([128, 4, J], F32, tag="ot4")
                nc.vector.tensor_scalar(ot4[:, b % 4, :], e1[:],
                                        k128[:, 0:1], None,
                                        op0=mybir.AluOpType.add)
                nc.vector.tensor_tensor(ot4[:, b % 4, :], ot4[:, b % 4, :],
                                        m2[:], op=mybir.AluOpType.add)
                if b % 4 == 3 or b == nbt - 1:
                    b0 = (b // 4) * 4
                    n4 = b - b0 + 1
                    dst_ap = out_d[b0 * 128:(b + 1) * 128, :].rearrange(
                        "(c p) j -> p c j", c=n4)
                    nc.sync.dma_start(dst_ap, ot4[:, 0:n4, :])

            early(0)
            for i in range(nbt + 1):
                if i < nbt:
                    dtab = dst_lo if i < nb_lo else dst_hi
                    di, _ = slices(i)
                    Dt = gp.tile([128, J, 256], FP16, tag="D")
                    nc.gpsimd.dma_gather(
                        out_ap=Dt[:], in_ap=dtab[:], idxs_ap=di,
                        num_idxs=B, num_idxs_reg=B, elem_size=256,
                        queue_num=i % 4,
                    )
                    st[i]['Dt'] = Dt
                if i + 1 < nbt:
                    early(i + 1)
                if i < nbt:
                    mid(i)
                if i >= 1:
                    late(i - 1)

    nc.compile()
    return nc


def _prep(inputs):
    src = np.asarray(inputs["src"]).astype(np.int64).ravel()
    dst = np.asarray(inputs["dst"]).astype(np.int64).ravel()
    s1 = np.asarray(inputs["s1"], np.float32)
    s2 = np.asarray(inputs["s2"], np.float32)
    p1 = np.asarray(inputs["p1"], np.float32)
    p2 = np.asarray(inputs["p2"], np.float32)
    x = np.asarray(inputs["x"], np.float32)
    w = np.asarray(inputs["w"], np.float32)
    w1 = np.asarray(inputs["w1"], np.float32).ravel()
    w2 = np.asarray(inputs["w2"], np.float32).ravel()

    E = src.shape[0]
    N = s1.shape[0]
    assert N == N_USERS
    epc = E // NCORES

    # src node table [N, 192]: [s1+s2 | x*w1 | w*w2]
    src_nodes = np.concatenate(
        [s1 + s2, x * w1[None, :], w * w2[None, :]], axis=1
    ).astype(np.float16)
    src_nodes_pad = np.vstack(
        [src_nodes, np.zeros((W, 192), np.float16)])
    # dst table [N, 256]: [p1+p2 | x | w | pad]
    dst_rows = np.concatenate(
        [p1 + p2, x, w, np.zeros((N, 64), np.float32)], axis=1
    ).astype(np.float16)

    order = np.argsort(src, kind='stable')

    per_core = []
    for c in range(NCORES):
        ids = order[c * epc:(c + 1) * epc]
        sc = src[ids]
        dc = dst[ids]
        halves = []
        for h in range(2):
            sel = np.flatnonzero((dc >= NHALF) == bool(h))
            s_h = sc[sel]
            d_h = dc[sel] - h * NHALF
            g_h = ids[sel]
            # batches: <=1024 edges, src range < 128
            batches = []
            i = 0
            n = len(s_h)
            while i < n:
                ws = s_h[i]
                j = min(i + B, n)
                j = min(j, int(np.searchsorted(s_h, ws + W, side='left')))
                batches.append((int(ws), i, j))
                i = j
            halves.append((s_h, d_h, g_h, batches))
        per_core.append(halves)

    nb_lo = max(len(per_core[c][0][3]) for c in range(NCORES))
    nb_hi = max(len(per_core[c][1][3]) for c in range(NCORES))
    nbt = nb_lo + nb_hi
    assert nbt <= 128, f"nbt={nbt} exceeds winrel row capacity"

    # packed layouts (see _build_program)
    nbt4 = -(-nbt // 4)
    win_tab = np.zeros((NCORES, 128, nbt * 192), np.float16)
    winrel = np.full((NCORES, nbt4, 4 * B), PADVAL, np.float16)
    dstidx = np.zeros((NCORES, 128, nbt * 64), np.int16)
    order_all = np.full((NCORES, nbt * B), -1, np.int64)

    for c in range(NCORES):
        for h in range(2):
            s_h, d_h, g_h, batches = per_core[c][h]
            b0 = 0 if h == 0 else nb_lo
            for k, (ws, i, j) in enumerate(batches):
                b = b0 + k
                n = j - i
                win_tab[c, :, b * 192:(b + 1) * 192] = \
                    src_nodes_pad[ws:ws + W]
                winrel[c, b // 4, (b % 4) * B:(b % 4) * B + n] = \
                    (s_h[i:j] - ws).astype(np.float16)
                dd = np.zeros(B, np.int64)
                dd[:n] = d_h[i:j]
                dstidx[c, :, b * 64:(b + 1) * 64] = _wrap_idx(dd)
                order_all[c, b * B:b * B + n] = g_h[i:j]

    # weights
    W_mlp = np.asarray(inputs["W_mlp"], np.float32)
    b_mlp = np.asarray(inputs["b_mlp"], np.float32).ravel()
    wL = np.asarray(inputs["wL"], np.float32).ravel()
    bL = float(np.asarray(inputs["bL"]).ravel()[0])
    b1 = float(np.asarray(inputs["b1"]).ravel()[0])
    b2 = float(np.asarray(inputs["b2"]).ravel()[0])

    wbd = np.zeros((128, 128), np.float16)
    wbd[:64, :64] = W_mlp.astype(np.float16)
    wbd[64:, 64:] = W_mlp.astype(np.float16)
    wlp = np.zeros((128, 2), np.float16)
    wlp[:64, 0] = wL.astype(np.float16)
    wlp[64:, 1] = wL.astype(np.float16)
    bm2 = np.concatenate([b_mlp, b_mlp]).astype(np.float32).reshape(128, 1)
    b3 = np.array([[bL, b1, b2]], np.float32)

    shared = dict(
        dst_lo=np.ascontiguousarray(dst_rows[:NHALF]),
        dst_hi=np.ascontiguousarray(dst_rows[NHALF:]),
        wbd=wbd, wlp=wlp, bm2=bm2, b3=b3,
    )
    return (nb_lo, nb_hi, epc, E, shared,
            win_tab, winrel, dstidx, order_all)


def _in_maps(shared, win_tab, winrel, dstidx):
    maps = []
    for c in range(NCORES):
        m = dict(shared)
        m["win_tab"] = win_tab[c]
        m["winrel"] = winrel[c]
        m["dstidx"] = dstidx[c]
        maps.append(m)
    return maps


def _unscramble(results, order_all, E, epc):
    nbt = order_all.shape[1] // B
    out = np.empty((E, 1), np.float32)
    for c in range(NCORES):
        oc = np.asarray(results[c]["out"], np.float32).reshape(nbt, 128, J)
        vals = oc.transpose(0, 2, 1).reshape(-1)  # list order b*B + j*128 + p
        ordv = order_all[c]
        valid = ordv >= 0
        out[ordv[valid], 0] = vals[valid]
    return out


def run(inputs, bcast='pool', **spmd_kwargs):
    from concourse.bass_utils import run_bass_kernel_spmd

    (nb_lo, nb_hi, epc, E, shared,
     win_tab, winrel, dstidx, order_all) = _prep(inputs)

    key = (nb_lo, nb_hi, bcast)
    if key not in _prog_cache:
        _prog_cache[key] = _build_program(nb_lo, nb_hi, bcast=bcast)
    nc = _prog_cache[key]

    maps = _in_maps(shared, win_tab, winrel, dstidx)
    res = run_bass_kernel_spmd(nc, maps, list(range(NCORES)), **spmd_kwargs)
    return _unscramble(res.results, order_all, E, epc), res


def kernel(**inputs) -> np.ndarray:
    out, _ = run(inputs)
    return out
